# revision 12
# baseline (speedup 1.0000x reference)
"""Additive (Bahdanau) attention on 8 TRN2 NeuronCores via Bass/Tile.

Problem: B=2, S=1024, H=512, A=128.
  wx = x @ w; ux = x @ u                       (B,S,A)
  e[b,i,j] = v . tanh(wx[b,i] + ux[b,j])      (B,S,S)
  attn = softmax_j(e)
  ctx[b] = sum_i sum_j attn[b,i,j] x[b,j]     (B,H)

Sharding: 8 cores = (batch b in 2) x (query-row block of 256 in 4).
Per-core algorithm (all A=128 on partitions):
  wxT (A, 256) = w.T @ xiT ;  uxT (A, S) = u.T @ xT       [PE]
  per query row i: th = tanh(uxT + bias=wxT[:,i])         [ACT, add fused into bias]
  e rows accumulate into PSUM (i on partitions) via a sliding selector
  matrix (v placed at column i of a zero (128,256) buffer) as matmul lhsT.
  Unstable softmax (|e| <= ||v||_1 ~ 9, safe in fp32): exp, rowsum, scale.
  colsum^T via matmul(lhsT=attn chunk, rhs=ones); ctx = colsum @ x.
Host sums the 4 partial ctx per batch and stitches attn blocks.
"""

import numpy as np

import concourse.bass as bass
import concourse.bacc as bacc
import concourse.mybir as mybir
from concourse.tile import TileContext
from concourse.bass_utils import run_bass_kernel_spmd

B, S, H, A = 2, 1024, 512, 128
NCORES = 8
IBLK = 256          # query rows per core
NGROUPS = IBLK // 128
KH = H // 128       # k-tiles over H
F32 = mybir.dt.float32
AF = mybir.ActivationFunctionType


def build_nc() -> bass.Bass:
    nc = bacc.Bacc()

    x_p = nc.declare_dram_parameter("x", [S, H], F32, isOutput=False)
    xT_p = nc.declare_dram_parameter("xT", [H, S], F32, isOutput=False)
    xiT_p = nc.declare_dram_parameter("xiT", [H, IBLK], F32, isOutput=False)
    w_p = nc.declare_dram_parameter("w", [H, A], F32, isOutput=False)
    u_p = nc.declare_dram_parameter("u", [H, A], F32, isOutput=False)
    zb_p = nc.declare_dram_parameter("zbuf", [A, 2 * A], F32, isOutput=False)
    attn_out = nc.declare_dram_parameter("attn_out", [IBLK, S], F32, isOutput=True)
    ctx_out = nc.declare_dram_parameter("ctx_out", [1, H], F32, isOutput=True)

    with TileContext(nc) as tc:
        with tc.tile_pool(name="const", bufs=1) as constp:
            w_sb = constp.tile([128, KH, A], F32)
            u_sb = constp.tile([128, KH, A], F32)
            xT_sb = constp.tile([128, KH, S], F32)
            xiT_sb = constp.tile([128, KH, IBLK], F32)
            xj_sb = constp.tile([128, S // 128, H], F32)
            zb_sb = constp.tile([128, 2 * A], F32)
            ones_sb = constp.tile([128, 1], F32)
            wxT_sb = constp.tile([128, IBLK], F32)
            uxT_sb = constp.tile([128, S], F32)
            csT_sb = constp.tile([128, S // 128], F32)
            ctx_sb = constp.tile([1, H], F32)

            nc.sync.dma_start(out=w_sb, in_=w_p.rearrange("(k p) a -> p k a", p=128))
            nc.sync.dma_start(out=u_sb, in_=u_p.rearrange("(k p) a -> p k a", p=128))
            nc.sync.dma_start(out=xT_sb, in_=xT_p.rearrange("(k p) j -> p k j", p=128))
            nc.sync.dma_start(
                out=xiT_sb, in_=xiT_p.rearrange("(k p) i -> p k i", p=128)
            )
            nc.sync.dma_start(out=xj_sb, in_=x_p.rearrange("(t p) h -> p t h", p=128))
            nc.sync.dma_start(out=zb_sb, in_=zb_p[:, :])
            nc.vector.memset(ones_sb, 1.0)

            with (
                tc.tile_pool(name="pre_ps", bufs=1, space="PSUM") as pps,
                tc.tile_pool(name="eps", bufs=2, space="PSUM") as eps,
                tc.tile_pool(name="csps", bufs=1, space="PSUM") as csps,
                tc.tile_pool(name="work", bufs=3) as work,
                tc.tile_pool(name="soft", bufs=2) as soft,
            ):
                # wxT (A, IBLK) and uxT (A, S) projections
                wx_ps = pps.tile([128, IBLK], F32)
                for k in range(KH):
                    nc.tensor.matmul(
                        wx_ps,
                        w_sb[:, k, :],
                        xiT_sb[:, k, :],
                        start=(k == 0),
                        stop=(k == KH - 1),
                    )
                nc.vector.tensor_copy(wxT_sb, wx_ps)
                for nj in range(2):
                    ux_ps = pps.tile([128, 512], F32)
                    for k in range(KH):
                        nc.tensor.matmul(
                            ux_ps,
                            u_sb[:, k, :],
                            xT_sb[:, k, nj * 512 : (nj + 1) * 512],
                            start=(k == 0),
                            stop=(k == KH - 1),
                        )
                    nc.vector.tensor_copy(uxT_sb[:, nj * 512 : (nj + 1) * 512], ux_ps)

                for g in range(NGROUPS):
                    e_ps0 = eps.tile([128, 512], F32, tag="e0")
                    e_ps1 = eps.tile([128, 512], F32, tag="e1")
                    for im in range(128):
                        i = g * 128 + im
                        th = work.tile([128, S], F32, tag="th")
                        nc.scalar.activation(
                            th, uxT_sb, AF.Tanh, bias=wxT_sb[:, i : i + 1]
                        )
                        nc.tensor.matmul(
                            e_ps0,
                            zb_sb[:, 128 - im : 256 - im],
                            th[:, 0:512],
                            start=(im == 0),
                            stop=(im == 127),
                        )
                        nc.tensor.matmul(
                            e_ps1,
                            zb_sb[:, 128 - im : 256 - im],
                            th[:, 512:1024],
                            start=(im == 0),
                            stop=(im == 127),
                        )
                    ea = soft.tile([128, S], F32, tag="ea")
                    rs = soft.tile([128, 1], F32, tag="rs")
                    rr = soft.tile([128, 1], F32, tag="rr")
                    at = soft.tile([128, S], F32, tag="at")
                    nc.scalar.activation(ea[:, 0:512], e_ps0, AF.Exp)
                    nc.scalar.activation(ea[:, 512:1024], e_ps1, AF.Exp)
                    nc.vector.reduce_sum(rs, ea, axis=mybir.AxisListType.X)
                    nc.vector.reciprocal(rr, rs)
                    nc.vector.tensor_scalar_mul(at, ea, rr)
                    nc.sync.dma_start(
                        out=attn_out[g * 128 : (g + 1) * 128, :], in_=at
                    )
                    # colsum^T for this group: csg[j, 0] = sum_i at[i, j]
                    csg_ps = csps.tile([128, S // 128], F32, tag="csg", bufs=2)
                    for t in range(S // 128):
                        nc.tensor.matmul(
                            csg_ps[:, t : t + 1],
                            at[:, t * 128 : (t + 1) * 128],
                            ones_sb,
                            start=True,
                            stop=True,
                        )
                    if g == 0:
                        nc.vector.tensor_copy(csT_sb, csg_ps)
                    else:
                        nc.vector.tensor_add(csT_sb, csT_sb, csg_ps)

                ctx_ps = eps.tile([1, H], F32, tag="e0")
                for t in range(S // 128):
                    nc.tensor.matmul(
                        ctx_ps,
                        csT_sb[:, t : t + 1],
                        xj_sb[:, t, :],
                        start=(t == 0),
                        stop=(t == S // 128 - 1),
                    )
                nc.vector.tensor_copy(ctx_sb, ctx_ps)
                nc.sync.dma_start(out=ctx_out[:, :], in_=ctx_sb)

    nc.finalize()
    return nc


_NC_CACHE = None
_RUN_KWARGS = {}
LAST_RESULTS = None


def _get_nc():
    global _NC_CACHE
    if _NC_CACHE is None:
        _NC_CACHE = build_nc()
    return _NC_CACHE


def kernel(lstm_output, w, u, v):
    lstm_output = np.ascontiguousarray(np.asarray(lstm_output, dtype=np.float32))
    w = np.ascontiguousarray(np.asarray(w, dtype=np.float32))
    u = np.ascontiguousarray(np.asarray(u, dtype=np.float32))
    v = np.asarray(v, dtype=np.float32)

    zbuf = np.zeros((A, 2 * A), np.float32)
    zbuf[:, 128] = v

    in_maps = []
    for c in range(NCORES):
        b, blk = divmod(c, 4)
        xb = lstm_output[b]
        in_maps.append(
            {
                "x": xb,
                "xT": np.ascontiguousarray(xb.T),
                "xiT": np.ascontiguousarray(xb[blk * IBLK : (blk + 1) * IBLK].T),
                "w": w,
                "u": u,
                "zbuf": zbuf,
            }
        )

    kres = run_bass_kernel_spmd(
        _get_nc(), in_maps, list(range(NCORES)), **_RUN_KWARGS
    )
    global LAST_RESULTS
    LAST_RESULTS = kres
    res = kres.results

    attn = np.empty((B, S, S), np.float32)
    ctx = np.zeros((B, H), np.float32)
    for c in range(NCORES):
        b, blk = divmod(c, 4)
        attn[b, blk * IBLK : (blk + 1) * IBLK] = res[c]["attn_out"]
        ctx[b] += res[c]["ctx_out"][0]
    return ctx, attn


# revision 19
# speedup vs baseline: 1.5195x; 1.5195x over previous
"""Additive (Bahdanau) attention on 8 TRN2 NeuronCores via Bass/Tile.

Problem: B=2, S=1024, H=512, A=128.
  wx = x @ w; ux = x @ u                       (B,S,A)
  e[b,i,j] = v . tanh(wx[b,i] + ux[b,j])      (B,S,S)
  attn = softmax_j(e)
  ctx[b] = sum_i sum_j attn[b,i,j] x[b,j]     (B,H)

Sharding: 8 cores = (batch b in 2) x (query-row block of 256 in 4).
Per-core algorithm (all A=128 on partitions):
  wxT (A, 256) = w.T @ xiT ;  uxT (A, S) = u.T @ xT       [PE]
  per query row i: th = tanh(uxT + bias=wxT[:,i])         [ACT, add fused into bias]
  e rows accumulate into PSUM (i on partitions) via a sliding selector
  matrix (v placed at column i of a zero (128,256) buffer) as matmul lhsT.
  Unstable softmax (|e| <= ||v||_1 ~ 9, safe in fp32): exp, rowsum, scale.
  colsum^T via matmul(lhsT=attn chunk, rhs=ones); ctx = colsum @ x.
Host sums the 4 partial ctx per batch and stitches attn blocks.
"""

import numpy as np

import concourse.bass as bass
import concourse.bacc as bacc
import concourse.mybir as mybir
from concourse.tile import TileContext
from concourse.bass_utils import run_bass_kernel_spmd

B, S, H, A = 2, 1024, 512, 128
NCORES = 8
IBLK = 256          # query rows per core
NGROUPS = IBLK // 128
KH = H // 128       # k-tiles over H
F32 = mybir.dt.float32
F32R = mybir.dt.float32r
AF = mybir.ActivationFunctionType


def build_nc() -> bass.Bass:
    nc = bacc.Bacc()

    x_p = nc.declare_dram_parameter("x", [S, H], F32, isOutput=False)
    xT_p = nc.declare_dram_parameter("xT", [H, S], F32, isOutput=False)
    xiT_p = nc.declare_dram_parameter("xiT", [H, IBLK], F32, isOutput=False)
    w_p = nc.declare_dram_parameter("w", [H, A], F32, isOutput=False)
    u_p = nc.declare_dram_parameter("u", [H, A], F32, isOutput=False)
    zb_p = nc.declare_dram_parameter("zbuf", [A, 2 * A], F32, isOutput=False)
    attn_out = nc.declare_dram_parameter("attn_out", [IBLK, S], F32, isOutput=True)
    ctx_out = nc.declare_dram_parameter("ctx_out", [1, H], F32, isOutput=True)

    with TileContext(nc) as tc:
        with tc.tile_pool(name="const", bufs=1) as constp:
            w_sb = constp.tile([128, KH, A], F32)
            u_sb = constp.tile([128, KH, A], F32)
            xT_sb = constp.tile([128, KH, S], F32)
            xiT_sb = constp.tile([128, KH, IBLK], F32)
            xj_sb = constp.tile([128, S // 128, H], F32)
            zb_sb = constp.tile([128, 2 * A], F32R)
            ones_sb = constp.tile([128, 1], F32)
            wxT_sb = constp.tile([128, IBLK], F32)
            uxT_sb = constp.tile([128, S], F32)
            csT_sb = constp.tile([128, S // 128], F32)
            ctx_sb = constp.tile([1, H], F32)

            nc.sync.dma_start(out=w_sb, in_=w_p.rearrange("(k p) a -> p k a", p=128))
            nc.sync.dma_start(out=u_sb, in_=u_p.rearrange("(k p) a -> p k a", p=128))
            nc.sync.dma_start(out=xT_sb, in_=xT_p.rearrange("(k p) j -> p k j", p=128))
            nc.sync.dma_start(
                out=xiT_sb, in_=xiT_p.rearrange("(k p) i -> p k i", p=128)
            )
            nc.sync.dma_start(out=xj_sb, in_=x_p.rearrange("(t p) h -> p t h", p=128))
            nc.sync.dma_start(out=zb_sb, in_=zb_p[:, :].bitcast(F32R))
            nc.vector.memset(ones_sb, 1.0)

            with (
                tc.tile_pool(name="pre_ps", bufs=1, space="PSUM") as pps,
                tc.tile_pool(name="eps", bufs=2, space="PSUM") as eps,
                tc.tile_pool(name="csps", bufs=1, space="PSUM") as csps,
                tc.tile_pool(name="work", bufs=3) as work,
                tc.tile_pool(name="soft", bufs=2) as soft,
            ):
                # wxT (A, IBLK) and uxT (A, S) projections
                wx_ps = pps.tile([128, IBLK], F32)
                for k in range(KH):
                    nc.tensor.matmul(
                        wx_ps,
                        w_sb[:, k, :],
                        xiT_sb[:, k, :],
                        start=(k == 0),
                        stop=(k == KH - 1),
                    )
                nc.vector.tensor_copy(wxT_sb, wx_ps)
                for nj in range(2):
                    ux_ps = pps.tile([128, 512], F32)
                    for k in range(KH):
                        nc.tensor.matmul(
                            ux_ps,
                            u_sb[:, k, :],
                            xT_sb[:, k, nj * 512 : (nj + 1) * 512],
                            start=(k == 0),
                            stop=(k == KH - 1),
                        )
                    nc.vector.tensor_copy(uxT_sb[:, nj * 512 : (nj + 1) * 512], ux_ps)

                for g in range(NGROUPS):
                    e_ps0 = eps.tile([128, 512], F32, tag="e0")
                    e_ps1 = eps.tile([128, 512], F32, tag="e1")
                    for im in range(128):
                        i = g * 128 + im
                        # float32r: single-pass fp32 matmul (1 cyc/row at N>=256)
                        th = work.tile([128, S], F32R, tag="th")
                        nc.scalar.activation(
                            th, uxT_sb, AF.Tanh, bias=wxT_sb[:, i : i + 1]
                        )
                        zr = zb_sb[:, 128 - im : 256 - im]
                        nc.tensor.matmul(
                            e_ps0,
                            zr,
                            th[:, 0:512],
                            start=(im == 0),
                            stop=(im == 127),
                        )
                        nc.tensor.matmul(
                            e_ps1,
                            zr,
                            th[:, 512:1024],
                            start=(im == 0),
                            stop=(im == 127),
                        )
                    ea = soft.tile([128, S], F32, tag="ea")
                    rs = soft.tile([128, 1], F32, tag="rs")
                    rr = soft.tile([128, 1], F32, tag="rr")
                    at = soft.tile([128, S], F32, tag="at")
                    nc.scalar.activation(ea[:, 0:512], e_ps0, AF.Exp)
                    nc.scalar.activation(ea[:, 512:1024], e_ps1, AF.Exp)
                    nc.vector.reduce_sum(rs, ea, axis=mybir.AxisListType.X)
                    nc.vector.reciprocal(rr, rs)
                    nc.vector.tensor_scalar_mul(at, ea, rr)
                    nc.sync.dma_start(
                        out=attn_out[g * 128 : (g + 1) * 128, :], in_=at
                    )
                    # colsum^T for this group: csg[j, 0] = sum_i at[i, j]
                    csg_ps = csps.tile([128, S // 128], F32, tag="csg", bufs=2)
                    for t in range(S // 128):
                        nc.tensor.matmul(
                            csg_ps[:, t : t + 1],
                            at[:, t * 128 : (t + 1) * 128],
                            ones_sb,
                            start=True,
                            stop=True,
                        )
                    if g == 0:
                        nc.vector.tensor_copy(csT_sb, csg_ps)
                    else:
                        nc.vector.tensor_add(csT_sb, csT_sb, csg_ps)

                ctx_ps = eps.tile([1, H], F32, tag="e0")
                for t in range(S // 128):
                    nc.tensor.matmul(
                        ctx_ps,
                        csT_sb[:, t : t + 1],
                        xj_sb[:, t, :],
                        start=(t == 0),
                        stop=(t == S // 128 - 1),
                    )
                nc.vector.tensor_copy(ctx_sb, ctx_ps)
                nc.sync.dma_start(out=ctx_out[:, :], in_=ctx_sb)

    nc.finalize()
    return nc


_NC_CACHE = None
_RUN_KWARGS = {}
LAST_RESULTS = None


def _get_nc():
    global _NC_CACHE
    if _NC_CACHE is None:
        _NC_CACHE = build_nc()
    return _NC_CACHE


def kernel(lstm_output, w, u, v):
    lstm_output = np.ascontiguousarray(np.asarray(lstm_output, dtype=np.float32))
    w = np.ascontiguousarray(np.asarray(w, dtype=np.float32))
    u = np.ascontiguousarray(np.asarray(u, dtype=np.float32))
    v = np.asarray(v, dtype=np.float32)

    zbuf = np.zeros((A, 2 * A), np.float32)
    zbuf[:, 128] = v

    in_maps = []
    for c in range(NCORES):
        b, blk = divmod(c, 4)
        xb = lstm_output[b]
        in_maps.append(
            {
                "x": xb,
                "xT": np.ascontiguousarray(xb.T),
                "xiT": np.ascontiguousarray(xb[blk * IBLK : (blk + 1) * IBLK].T),
                "w": w,
                "u": u,
                "zbuf": zbuf,
            }
        )

    kres = run_bass_kernel_spmd(
        _get_nc(), in_maps, list(range(NCORES)), **_RUN_KWARGS
    )
    global LAST_RESULTS
    LAST_RESULTS = kres
    res = kres.results

    attn = np.empty((B, S, S), np.float32)
    ctx = np.zeros((B, H), np.float32)
    for c in range(NCORES):
        b, blk = divmod(c, 4)
        attn[b, blk * IBLK : (blk + 1) * IBLK] = res[c]["attn_out"]
        ctx[b] += res[c]["ctx_out"][0]
    return ctx, attn


# revision 20
# speedup vs baseline: 1.7159x; 1.1293x over previous
"""Additive (Bahdanau) attention on 8 TRN2 NeuronCores via Bass/Tile.

Problem: B=2, S=1024, H=512, A=128.
  wx = x @ w; ux = x @ u                       (B,S,A)
  e[b,i,j] = v . tanh(wx[b,i] + ux[b,j])      (B,S,S)
  attn = softmax_j(e)
  ctx[b] = sum_i sum_j attn[b,i,j] x[b,j]     (B,H)

Sharding: 8 cores = (batch b in 2) x (query-row block of 256 in 4).
Per-core algorithm (all A=128 on partitions):
  wxT (A, 256) = w.T @ xiT ;  uxT (A, S) = u.T @ xT       [PE]
  per query row i: th = tanh(uxT + bias=wxT[:,i])         [ACT, add fused into bias]
  e rows accumulate into PSUM (i on partitions) via a sliding selector
  matrix (v placed at column i of a zero (128,256) buffer) as matmul lhsT.
  Unstable softmax (|e| <= ||v||_1 ~ 9, safe in fp32): exp, rowsum, scale.
  colsum^T via matmul(lhsT=attn chunk, rhs=ones); ctx = colsum @ x.
Host sums the 4 partial ctx per batch and stitches attn blocks.
"""

import numpy as np

import concourse.bass as bass
import concourse.bacc as bacc
import concourse.mybir as mybir
from concourse.tile import TileContext
from concourse.bass_utils import run_bass_kernel_spmd

B, S, H, A = 2, 1024, 512, 128
NCORES = 8
IBLK = 256          # query rows per core
NGROUPS = IBLK // 128
KH = H // 128       # k-tiles over H
F32 = mybir.dt.float32
F32R = mybir.dt.float32r
AF = mybir.ActivationFunctionType


def build_nc() -> bass.Bass:
    nc = bacc.Bacc()

    x_p = nc.declare_dram_parameter("x", [S, H], F32, isOutput=False)
    xT_p = nc.declare_dram_parameter("xT", [H, S], F32, isOutput=False)
    xiT_p = nc.declare_dram_parameter("xiT", [H, IBLK], F32, isOutput=False)
    w_p = nc.declare_dram_parameter("w", [H, A], F32, isOutput=False)
    u_p = nc.declare_dram_parameter("u", [H, A], F32, isOutput=False)
    zb_p = nc.declare_dram_parameter("zbuf", [A, 2 * A], F32, isOutput=False)
    attn_out = nc.declare_dram_parameter("attn_out", [IBLK, S], F32, isOutput=True)
    ctx_out = nc.declare_dram_parameter("ctx_out", [1, H], F32, isOutput=True)

    with TileContext(nc) as tc:
        with tc.tile_pool(name="const", bufs=1) as constp:
            w_sb = constp.tile([128, KH, A], F32)
            u_sb = constp.tile([128, KH, A], F32)
            xT_sb = constp.tile([128, KH, S], F32)
            xiT_sb = constp.tile([128, KH, IBLK], F32)
            xj_sb = constp.tile([128, S // 128, H], F32)
            zb_sb = constp.tile([128, 2 * A], F32R)
            ones_sb = constp.tile([128, 1], F32)
            wxT_sb = constp.tile([128, IBLK], F32)
            uxT_sb = constp.tile([128, S], F32)
            csT_sb = constp.tile([128, S // 128], F32)
            ctx_sb = constp.tile([1, H], F32)

            nc.sync.dma_start(out=w_sb, in_=w_p.rearrange("(k p) a -> p k a", p=128))
            nc.sync.dma_start(out=u_sb, in_=u_p.rearrange("(k p) a -> p k a", p=128))
            nc.sync.dma_start(out=xT_sb, in_=xT_p.rearrange("(k p) j -> p k j", p=128))
            nc.sync.dma_start(
                out=xiT_sb, in_=xiT_p.rearrange("(k p) i -> p k i", p=128)
            )
            nc.sync.dma_start(out=xj_sb, in_=x_p.rearrange("(t p) h -> p t h", p=128))
            nc.sync.dma_start(out=zb_sb, in_=zb_p[:, :].bitcast(F32R))
            nc.vector.memset(ones_sb, 1.0)

            with (
                tc.tile_pool(name="pre_ps", bufs=1, space="PSUM") as pps,
                tc.tile_pool(name="eps", bufs=2, space="PSUM") as eps,
                tc.tile_pool(name="csps", bufs=1, space="PSUM") as csps,
                tc.tile_pool(name="work", bufs=3) as work,
                tc.tile_pool(name="soft", bufs=2) as soft,
            ):
                # wxT (A, IBLK) and uxT (A, S) projections
                wx_ps = pps.tile([128, IBLK], F32)
                for k in range(KH):
                    nc.tensor.matmul(
                        wx_ps,
                        w_sb[:, k, :],
                        xiT_sb[:, k, :],
                        start=(k == 0),
                        stop=(k == KH - 1),
                    )
                nc.vector.tensor_copy(wxT_sb, wx_ps)
                for nj in range(2):
                    ux_ps = pps.tile([128, 512], F32)
                    for k in range(KH):
                        nc.tensor.matmul(
                            ux_ps,
                            u_sb[:, k, :],
                            xT_sb[:, k, nj * 512 : (nj + 1) * 512],
                            start=(k == 0),
                            stop=(k == KH - 1),
                        )
                    nc.vector.tensor_copy(uxT_sb[:, nj * 512 : (nj + 1) * 512], ux_ps)

                R = 4  # query rows per ACT call (amortizes ACT per-inst overhead)
                for g in range(NGROUPS):
                    e_ps0 = eps.tile([128, 512], F32, tag="e0")
                    e_ps1 = eps.tile([128, 512], F32, tag="e1")
                    for im in range(0, 128, R):
                        i = g * 128 + im
                        # DVE builds the biased inputs (2 elem/cyc tensor_scalar),
                        # ACT does one wide tanh, PE reduces via fp32r selector
                        # matmuls accumulating e rows at partition i.
                        bq = work.tile([128, R, S], F32, tag="bq")
                        for r in range(R):
                            nc.vector.tensor_scalar_add(
                                bq[:, r, :], uxT_sb, wxT_sb[:, i + r : i + r + 1]
                            )
                        th = work.tile([128, R, S], F32R, tag="th")
                        nc.scalar.activation(th, bq, AF.Tanh)
                        for r in range(R):
                            zr = zb_sb[:, 128 - (im + r) : 256 - (im + r)]
                            nc.tensor.matmul(
                                e_ps0,
                                zr,
                                th[:, r, 0:512],
                                start=(im + r == 0),
                                stop=(im + r == 127),
                            )
                            nc.tensor.matmul(
                                e_ps1,
                                zr,
                                th[:, r, 512:1024],
                                start=(im + r == 0),
                                stop=(im + r == 127),
                            )
                    ea = soft.tile([128, S], F32, tag="ea")
                    rs = soft.tile([128, 1], F32, tag="rs")
                    rr = soft.tile([128, 1], F32, tag="rr")
                    at = soft.tile([128, S], F32, tag="at")
                    nc.scalar.activation(ea[:, 0:512], e_ps0, AF.Exp)
                    nc.scalar.activation(ea[:, 512:1024], e_ps1, AF.Exp)
                    nc.vector.reduce_sum(rs, ea, axis=mybir.AxisListType.X)
                    nc.vector.reciprocal(rr, rs)
                    nc.vector.tensor_scalar_mul(at, ea, rr)
                    nc.sync.dma_start(
                        out=attn_out[g * 128 : (g + 1) * 128, :], in_=at
                    )
                    # colsum^T for this group: csg[j, 0] = sum_i at[i, j]
                    csg_ps = csps.tile([128, S // 128], F32, tag="csg", bufs=2)
                    for t in range(S // 128):
                        nc.tensor.matmul(
                            csg_ps[:, t : t + 1],
                            at[:, t * 128 : (t + 1) * 128],
                            ones_sb,
                            start=True,
                            stop=True,
                        )
                    if g == 0:
                        nc.vector.tensor_copy(csT_sb, csg_ps)
                    else:
                        nc.vector.tensor_add(csT_sb, csT_sb, csg_ps)

                ctx_ps = eps.tile([1, H], F32, tag="e0")
                for t in range(S // 128):
                    nc.tensor.matmul(
                        ctx_ps,
                        csT_sb[:, t : t + 1],
                        xj_sb[:, t, :],
                        start=(t == 0),
                        stop=(t == S // 128 - 1),
                    )
                nc.vector.tensor_copy(ctx_sb, ctx_ps)
                nc.sync.dma_start(out=ctx_out[:, :], in_=ctx_sb)

    nc.finalize()
    return nc


_NC_CACHE = None
_RUN_KWARGS = {}
LAST_RESULTS = None


def _get_nc():
    global _NC_CACHE
    if _NC_CACHE is None:
        _NC_CACHE = build_nc()
    return _NC_CACHE


def kernel(lstm_output, w, u, v):
    lstm_output = np.ascontiguousarray(np.asarray(lstm_output, dtype=np.float32))
    w = np.ascontiguousarray(np.asarray(w, dtype=np.float32))
    u = np.ascontiguousarray(np.asarray(u, dtype=np.float32))
    v = np.asarray(v, dtype=np.float32)

    zbuf = np.zeros((A, 2 * A), np.float32)
    zbuf[:, 128] = v

    in_maps = []
    for c in range(NCORES):
        b, blk = divmod(c, 4)
        xb = lstm_output[b]
        in_maps.append(
            {
                "x": xb,
                "xT": np.ascontiguousarray(xb.T),
                "xiT": np.ascontiguousarray(xb[blk * IBLK : (blk + 1) * IBLK].T),
                "w": w,
                "u": u,
                "zbuf": zbuf,
            }
        )

    kres = run_bass_kernel_spmd(
        _get_nc(), in_maps, list(range(NCORES)), **_RUN_KWARGS
    )
    global LAST_RESULTS
    LAST_RESULTS = kres
    res = kres.results

    attn = np.empty((B, S, S), np.float32)
    ctx = np.zeros((B, H), np.float32)
    for c in range(NCORES):
        b, blk = divmod(c, 4)
        attn[b, blk * IBLK : (blk + 1) * IBLK] = res[c]["attn_out"]
        ctx[b] += res[c]["ctx_out"][0]
    return ctx, attn


# revision 23
# speedup vs baseline: 1.7475x; 1.0184x over previous
"""Additive (Bahdanau) attention on 8 TRN2 NeuronCores via Bass/Tile.

Problem: B=2, S=1024, H=512, A=128.
  wx = x @ w; ux = x @ u                       (B,S,A)
  e[b,i,j] = v . tanh(wx[b,i] + ux[b,j])      (B,S,S)
  attn = softmax_j(e)
  ctx[b] = sum_i sum_j attn[b,i,j] x[b,j]     (B,H)

Sharding: 8 cores = (batch b in 2) x (query-row block of 256 in 4).
Per-core algorithm (all A=128 on partitions):
  wxT (A, 256) = w.T @ xiT ;  uxT (A, S) = u.T @ xT       [PE]
  per query row i: th = tanh(uxT + bias=wxT[:,i])         [ACT, add fused into bias]
  e rows accumulate into PSUM (i on partitions) via a sliding selector
  matrix (v placed at column i of a zero (128,256) buffer) as matmul lhsT.
  Unstable softmax (|e| <= ||v||_1 ~ 9, safe in fp32): exp, rowsum, scale.
  colsum^T via matmul(lhsT=attn chunk, rhs=ones); ctx = colsum @ x.
Host sums the 4 partial ctx per batch and stitches attn blocks.
"""

import numpy as np

import concourse.bass as bass
import concourse.bacc as bacc
import concourse.mybir as mybir
from concourse.tile import TileContext
from concourse.bass_utils import run_bass_kernel_spmd

B, S, H, A = 2, 1024, 512, 128
NCORES = 8
IBLK = 256          # query rows per core
NGROUPS = IBLK // 128
KH = H // 128       # k-tiles over H
F32 = mybir.dt.float32
F32R = mybir.dt.float32r
AF = mybir.ActivationFunctionType


def build_nc() -> bass.Bass:
    nc = bacc.Bacc()

    x_p = nc.declare_dram_parameter("x", [S, H], F32, isOutput=False)
    xT_p = nc.declare_dram_parameter("xT", [H, S], F32, isOutput=False)
    xiT_p = nc.declare_dram_parameter("xiT", [H, IBLK], F32, isOutput=False)
    w_p = nc.declare_dram_parameter("w", [H, A], F32, isOutput=False)
    u_p = nc.declare_dram_parameter("u", [H, A], F32, isOutput=False)
    zb_p = nc.declare_dram_parameter("zbuf", [A, 2 * A], F32, isOutput=False)
    attn_out = nc.declare_dram_parameter("attn_out", [IBLK, S], F32, isOutput=True)
    ctx_out = nc.declare_dram_parameter("ctx_out", [1, H], F32, isOutput=True)

    with TileContext(nc) as tc:
        with tc.tile_pool(name="const", bufs=1) as constp:
            w_sb = constp.tile([128, KH, A], F32)
            u_sb = constp.tile([128, KH, A], F32)
            xT_sb = constp.tile([128, KH, S], F32)
            xiT_sb = constp.tile([128, KH, IBLK], F32)
            xj_sb = constp.tile([128, S // 128, H], F32R)
            zb_sb = constp.tile([128, 2 * A], F32R)
            warm_sb = constp.tile([128, 1], F32)
            wxT_sb = constp.tile([128, IBLK], F32)
            uxT_sb = constp.tile([128, S], F32)
            csT_sb = constp.tile([128, S // 128], F32R)
            ctx_sb = constp.tile([1, H], F32)

            # prefetch the ACT exp/tanh table while DMAs run
            nc.vector.memset(warm_sb, 0.0)
            nc.scalar.activation(warm_sb, warm_sb, AF.Tanh)

            nc.sync.dma_start(out=xT_sb, in_=xT_p.rearrange("(k p) j -> p k j", p=128))
            nc.sync.dma_start(out=u_sb, in_=u_p.rearrange("(k p) a -> p k a", p=128))
            nc.sync.dma_start(
                out=xiT_sb, in_=xiT_p.rearrange("(k p) i -> p k i", p=128)
            )
            nc.sync.dma_start(out=w_sb, in_=w_p.rearrange("(k p) a -> p k a", p=128))
            nc.sync.dma_start(out=zb_sb, in_=zb_p[:, :].bitcast(F32R))
            nc.sync.dma_start(
                out=xj_sb, in_=x_p.rearrange("(t p) h -> p t h", p=128).bitcast(F32R)
            )

            with (
                tc.tile_pool(name="pre_ps", bufs=1, space="PSUM") as pps,
                tc.tile_pool(name="eps", bufs=2, space="PSUM") as eps,
                tc.tile_pool(name="csps", bufs=1, space="PSUM") as csps,
                tc.tile_pool(name="work", bufs=2) as work,
                tc.tile_pool(name="soft", bufs=2) as soft,
            ):
                # uxT (A, S) then wxT (A, IBLK) projections (ux on the
                # critical path to the first tanh quad)
                for nj in range(2):
                    ux_ps = pps.tile([128, 512], F32)
                    for k in range(KH):
                        nc.tensor.matmul(
                            ux_ps,
                            u_sb[:, k, :],
                            xT_sb[:, k, nj * 512 : (nj + 1) * 512],
                            start=(k == 0),
                            stop=(k == KH - 1),
                        )
                    nc.vector.tensor_copy(uxT_sb[:, nj * 512 : (nj + 1) * 512], ux_ps)
                wx_ps = pps.tile([128, IBLK], F32)
                for k in range(KH):
                    nc.tensor.matmul(
                        wx_ps,
                        w_sb[:, k, :],
                        xiT_sb[:, k, :],
                        start=(k == 0),
                        stop=(k == KH - 1),
                    )
                nc.vector.tensor_copy(wxT_sb, wx_ps)

                R = 8  # query rows per ACT call (amortizes ACT per-inst overhead)
                for g in range(NGROUPS):
                    e_ps0 = eps.tile([128, 512], F32, tag="e0")
                    e_ps1 = eps.tile([128, 512], F32, tag="e1")
                    for im in range(0, 128, R):
                        i = g * 128 + im
                        # DVE builds the biased inputs (2 elem/cyc tensor_scalar),
                        # ACT does one wide tanh, PE reduces via fp32r selector
                        # matmuls accumulating e rows at partition i.
                        bq = work.tile([128, R, S], F32, tag="bq")
                        for r in range(R):
                            nc.vector.tensor_scalar_add(
                                bq[:, r, :], uxT_sb, wxT_sb[:, i + r : i + r + 1]
                            )
                        th = work.tile([128, R, S], F32R, tag="th")
                        nc.scalar.activation(th, bq, AF.Tanh)
                        for r in range(R):
                            zr = zb_sb[:, 128 - (im + r) : 256 - (im + r)]
                            nc.tensor.matmul(
                                e_ps0,
                                zr,
                                th[:, r, 0:512],
                                start=(im + r == 0),
                                stop=(im + r == 127),
                            )
                            nc.tensor.matmul(
                                e_ps1,
                                zr,
                                th[:, r, 512:1024],
                                start=(im + r == 0),
                                stop=(im + r == 127),
                            )
                    at = soft.tile([128, S], F32, tag="at")
                    rs0 = soft.tile([128, 1], F32, tag="rs0")
                    rs1 = soft.tile([128, 1], F32, tag="rs1")
                    rs = soft.tile([128, 1], F32, tag="rs")
                    rr = soft.tile([128, 1], F32, tag="rr")
                    # exp with fused row-sum (accum_out)
                    nc.scalar.activation(at[:, 0:512], e_ps0, AF.Exp, accum_out=rs0)
                    nc.scalar.activation(
                        at[:, 512:1024], e_ps1, AF.Exp, accum_out=rs1
                    )
                    nc.vector.tensor_add(rs, rs0, rs1)
                    nc.vector.reciprocal(rr, rs)
                    # colsum^T of the normalized attn, using the unnormalized
                    # exp as lhsT and rr as rhs: csg[j] = sum_i exp[i,j]*rr[i]
                    csg_ps = csps.tile([128, S // 128], F32, tag="csg", bufs=2)
                    for t in range(S // 128):
                        nc.tensor.matmul(
                            csg_ps[:, t : t + 1],
                            at[:, t * 128 : (t + 1) * 128],
                            rr,
                            start=True,
                            stop=True,
                        )
                    # normalize in place, then store
                    nc.vector.tensor_scalar_mul(at, at, rr)
                    nc.sync.dma_start(
                        out=attn_out[g * 128 : (g + 1) * 128, :], in_=at
                    )
                    if g == 0:
                        nc.vector.tensor_copy(csT_sb, csg_ps)
                    else:
                        nc.vector.tensor_add(csT_sb, csT_sb, csg_ps)

                ctx_ps = eps.tile([1, H], F32, tag="e0")
                for t in range(S // 128):
                    nc.tensor.matmul(
                        ctx_ps,
                        csT_sb[:, t : t + 1],
                        xj_sb[:, t, :],
                        start=(t == 0),
                        stop=(t == S // 128 - 1),
                    )
                nc.vector.tensor_copy(ctx_sb, ctx_ps)
                nc.sync.dma_start(out=ctx_out[:, :], in_=ctx_sb)

    nc.finalize()
    return nc


_NC_CACHE = None
_RUN_KWARGS = {}
LAST_RESULTS = None


def _get_nc():
    global _NC_CACHE
    if _NC_CACHE is None:
        _NC_CACHE = build_nc()
    return _NC_CACHE


def kernel(lstm_output, w, u, v):
    lstm_output = np.ascontiguousarray(np.asarray(lstm_output, dtype=np.float32))
    w = np.ascontiguousarray(np.asarray(w, dtype=np.float32))
    u = np.ascontiguousarray(np.asarray(u, dtype=np.float32))
    v = np.asarray(v, dtype=np.float32)

    zbuf = np.zeros((A, 2 * A), np.float32)
    zbuf[:, 128] = v

    in_maps = []
    for c in range(NCORES):
        b, blk = divmod(c, 4)
        xb = lstm_output[b]
        in_maps.append(
            {
                "x": xb,
                "xT": np.ascontiguousarray(xb.T),
                "xiT": np.ascontiguousarray(xb[blk * IBLK : (blk + 1) * IBLK].T),
                "w": w,
                "u": u,
                "zbuf": zbuf,
            }
        )

    kres = run_bass_kernel_spmd(
        _get_nc(), in_maps, list(range(NCORES)), **_RUN_KWARGS
    )
    global LAST_RESULTS
    LAST_RESULTS = kres
    res = kres.results

    attn = np.empty((B, S, S), np.float32)
    ctx = np.zeros((B, H), np.float32)
    for c in range(NCORES):
        b, blk = divmod(c, 4)
        attn[b, blk * IBLK : (blk + 1) * IBLK] = res[c]["attn_out"]
        ctx[b] += res[c]["ctx_out"][0]
    return ctx, attn


# revision 26
# speedup vs baseline: 1.8270x; 1.0455x over previous
"""Additive (Bahdanau) attention on 8 TRN2 NeuronCores via Bass/Tile.

Problem: B=2, S=1024, H=512, A=128.
  wx = x @ w; ux = x @ u                       (B,S,A)
  e[b,i,j] = v . tanh(wx[b,i] + ux[b,j])      (B,S,S)
  attn = softmax_j(e)
  ctx[b] = sum_i sum_j attn[b,i,j] x[b,j]     (B,H)

Sharding: 8 cores = (batch b in 2) x (query-row block of 256 in 4).
Per-core algorithm (all A=128 on partitions):
  wxT (A, 256) = w.T @ xiT ;  uxT (A, S) = u.T @ xT       [PE]
  per query row i: th = tanh(uxT + bias=wxT[:,i])         [ACT, add fused into bias]
  e rows accumulate into PSUM (i on partitions) via a sliding selector
  matrix (v placed at column i of a zero (128,256) buffer) as matmul lhsT.
  Unstable softmax (|e| <= ||v||_1 ~ 9, safe in fp32): exp, rowsum, scale.
  colsum^T via matmul(lhsT=attn chunk, rhs=ones); ctx = colsum @ x.
Host sums the 4 partial ctx per batch and stitches attn blocks.
"""

import numpy as np

import concourse.bass as bass
import concourse.bacc as bacc
import concourse.mybir as mybir
from concourse.tile import TileContext
from concourse.bass_utils import run_bass_kernel_spmd

B, S, H, A = 2, 1024, 512, 128
NCORES = 8
IBLK = 256          # query rows per core
NGROUPS = IBLK // 128
KH = H // 128       # k-tiles over H
F32 = mybir.dt.float32
F32R = mybir.dt.float32r
AF = mybir.ActivationFunctionType


def build_nc() -> bass.Bass:
    nc = bacc.Bacc()

    x_p = nc.declare_dram_parameter("x", [S, H], F32, isOutput=False)
    xT_p = nc.declare_dram_parameter("xT", [H, S], F32, isOutput=False)
    xiT_p = nc.declare_dram_parameter("xiT", [H, IBLK], F32, isOutput=False)
    w_p = nc.declare_dram_parameter("w", [H, A], F32, isOutput=False)
    u_p = nc.declare_dram_parameter("u", [H, A], F32, isOutput=False)
    zb_p = nc.declare_dram_parameter("zbuf", [A, 2 * A], F32, isOutput=False)
    attn_out = nc.declare_dram_parameter("attn_out", [IBLK, S], F32, isOutput=True)
    ctx_out = nc.declare_dram_parameter("ctx_out", [1, H], F32, isOutput=True)

    with TileContext(nc) as tc:
        with tc.tile_pool(name="const", bufs=1) as constp:
            w_sb = constp.tile([128, KH, A], F32R)
            u_sb = constp.tile([128, KH, A], F32R)
            xT_sb = constp.tile([128, KH, S], F32R)
            xiT_sb = constp.tile([128, KH, IBLK], F32R)
            xj_sb = constp.tile([128, S // 128, H], F32R)
            zb_sb = constp.tile([128, 2 * A], F32R)
            warm_sb = constp.tile([128, 1], F32)
            wxT_sb = constp.tile([128, IBLK], F32)
            uxT_sb = constp.tile([128, S], F32)
            csT_sb = constp.tile([128, S // 128], F32R)
            ctx_sb = constp.tile([1, H], F32)

            # prefetch the ACT exp/tanh table while DMAs run
            nc.vector.memset(warm_sb, 0.0)
            nc.scalar.activation(warm_sb, warm_sb, AF.Tanh)

            nc.sync.dma_start(
                out=u_sb, in_=u_p.rearrange("(k p) a -> p k a", p=128).bitcast(F32R)
            )
            xT_r = xT_p.rearrange("(k p) j -> k p j", p=128).bitcast(F32R)
            for k in range(KH):
                nc.sync.dma_start(out=xT_sb[:, k, :], in_=xT_r[k])
            nc.sync.dma_start(
                out=xiT_sb,
                in_=xiT_p.rearrange("(k p) i -> p k i", p=128).bitcast(F32R),
            )
            nc.sync.dma_start(
                out=w_sb, in_=w_p.rearrange("(k p) a -> p k a", p=128).bitcast(F32R)
            )
            nc.sync.dma_start(out=zb_sb, in_=zb_p[:, :].bitcast(F32R))
            nc.sync.dma_start(
                out=xj_sb, in_=x_p.rearrange("(t p) h -> p t h", p=128).bitcast(F32R)
            )

            with (
                tc.tile_pool(name="pre_ps", bufs=1, space="PSUM") as pps,
                tc.tile_pool(name="eps", bufs=2, space="PSUM") as eps,
                tc.tile_pool(name="csps", bufs=1, space="PSUM") as csps,
                tc.tile_pool(name="work", bufs=2) as work,
                tc.tile_pool(name="soft", bufs=2) as soft,
            ):
                # uxT (A, S) then wxT (A, IBLK) projections (ux on the
                # critical path to the first tanh quad)
                for nj in range(2):
                    ux_ps = pps.tile([128, 512], F32)
                    for k in range(KH):
                        nc.tensor.matmul(
                            ux_ps,
                            u_sb[:, k, :],
                            xT_sb[:, k, nj * 512 : (nj + 1) * 512],
                            start=(k == 0),
                            stop=(k == KH - 1),
                        )
                    nc.vector.tensor_copy(uxT_sb[:, nj * 512 : (nj + 1) * 512], ux_ps)
                wx_ps = pps.tile([128, IBLK], F32)
                for k in range(KH):
                    nc.tensor.matmul(
                        wx_ps,
                        w_sb[:, k, :],
                        xiT_sb[:, k, :],
                        start=(k == 0),
                        stop=(k == KH - 1),
                    )
                nc.vector.tensor_copy(wxT_sb, wx_ps)

                R = 8  # query rows per ACT call (amortizes ACT per-inst overhead)

                def softmax_tail(g, e_ps0, e_ps1):
                    at = soft.tile([128, S], F32, tag="at", name="at")
                    rs0 = soft.tile([128, 1], F32, tag="rs0", name="rs0")
                    rs1 = soft.tile([128, 1], F32, tag="rs1", name="rs1")
                    rs = soft.tile([128, 1], F32, tag="rs", name="rs")
                    rr = soft.tile([128, 1], F32, tag="rr", name="rr")
                    # exp with fused row-sum (accum_out)
                    nc.scalar.activation(at[:, 0:512], e_ps0, AF.Exp, accum_out=rs0)
                    nc.scalar.activation(
                        at[:, 512:1024], e_ps1, AF.Exp, accum_out=rs1
                    )
                    nc.vector.tensor_add(rs, rs0, rs1)
                    nc.vector.reciprocal(rr, rs)
                    # colsum^T of the normalized attn from the unnormalized
                    # exp: csg[j] = sum_i exp[i,j]*rr[i]
                    csg_ps = csps.tile(
                        [128, S // 128], F32, tag="csg", bufs=2, name="csg"
                    )
                    for t in range(S // 128):
                        nc.tensor.matmul(
                            csg_ps[:, t : t + 1],
                            at[:, t * 128 : (t + 1) * 128],
                            rr,
                            start=True,
                            stop=True,
                        )
                    # normalize in place, then store
                    nc.vector.tensor_scalar_mul(at, at, rr)
                    nc.sync.dma_start(
                        out=attn_out[g * 128 : (g + 1) * 128, :], in_=at
                    )
                    if g == 0:
                        nc.vector.tensor_copy(csT_sb, csg_ps)
                    else:
                        nc.vector.tensor_add(csT_sb, csT_sb, csg_ps)

                pending = None
                for g in range(NGROUPS):
                    e_ps0 = eps.tile([128, 512], F32, tag="e0", name="e_ps0")
                    e_ps1 = eps.tile([128, 512], F32, tag="e1", name="e_ps1")
                    if g == NGROUPS - 1:
                        # shrink the final quads: less PE drain after last tanh
                        quads = [8] * 15 + [4, 2, 2]
                    else:
                        quads = [8] * 16
                    im = 0
                    for qi, rq in enumerate(quads):
                        i = g * 128 + im
                        # DVE builds the biased inputs (2 elem/cyc
                        # tensor_scalar), ACT does one wide tanh, PE reduces
                        # via fp32r selector matmuls accumulating e rows at
                        # partition i.
                        bq = work.tile([128, rq, S], F32, tag="bq", name="bq")
                        for r in range(rq):
                            nc.vector.tensor_scalar_add(
                                bq[:, r, :], uxT_sb, wxT_sb[:, i + r : i + r + 1]
                            )
                        th = work.tile([128, rq, S], F32R, tag="th", name="th")
                        nc.scalar.activation(th, bq, AF.Tanh)
                        for r in range(rq):
                            zr = zb_sb[:, 128 - (im + r) : 256 - (im + r)]
                            nc.tensor.matmul(
                                e_ps0,
                                zr,
                                th[:, r, 0:512],
                                start=(im + r == 0),
                                stop=(im + r == 127),
                            )
                            nc.tensor.matmul(
                                e_ps1,
                                zr,
                                th[:, r, 512:1024],
                                start=(im + r == 0),
                                stop=(im + r == 127),
                            )
                        im += rq
                        if qi == 1 and pending is not None:
                            pending()
                            pending = None
                    g_, p0_, p1_ = g, e_ps0, e_ps1
                    pending = lambda g=g_, a=p0_, b=p1_: softmax_tail(g, a, b)
                pending()

                ctx_ps = eps.tile([1, H], F32, tag="e0")
                for t in range(S // 128):
                    nc.tensor.matmul(
                        ctx_ps,
                        csT_sb[:, t : t + 1],
                        xj_sb[:, t, :],
                        start=(t == 0),
                        stop=(t == S // 128 - 1),
                    )
                nc.vector.tensor_copy(ctx_sb, ctx_ps)
                nc.sync.dma_start(out=ctx_out[:, :], in_=ctx_sb)

    nc.finalize()
    return nc


_NC_CACHE = None
_RUN_KWARGS = {}
LAST_RESULTS = None


def _get_nc():
    global _NC_CACHE
    if _NC_CACHE is None:
        _NC_CACHE = build_nc()
    return _NC_CACHE


def kernel(lstm_output, w, u, v):
    lstm_output = np.ascontiguousarray(np.asarray(lstm_output, dtype=np.float32))
    w = np.ascontiguousarray(np.asarray(w, dtype=np.float32))
    u = np.ascontiguousarray(np.asarray(u, dtype=np.float32))
    v = np.asarray(v, dtype=np.float32)

    zbuf = np.zeros((A, 2 * A), np.float32)
    zbuf[:, 128] = v

    in_maps = []
    for c in range(NCORES):
        b, blk = divmod(c, 4)
        xb = lstm_output[b]
        in_maps.append(
            {
                "x": xb,
                "xT": np.ascontiguousarray(xb.T),
                "xiT": np.ascontiguousarray(xb[blk * IBLK : (blk + 1) * IBLK].T),
                "w": w,
                "u": u,
                "zbuf": zbuf,
            }
        )

    kres = run_bass_kernel_spmd(
        _get_nc(), in_maps, list(range(NCORES)), **_RUN_KWARGS
    )
    global LAST_RESULTS
    LAST_RESULTS = kres
    res = kres.results

    attn = np.empty((B, S, S), np.float32)
    ctx = np.zeros((B, H), np.float32)
    for c in range(NCORES):
        b, blk = divmod(c, 4)
        attn[b, blk * IBLK : (blk + 1) * IBLK] = res[c]["attn_out"]
        ctx[b] += res[c]["ctx_out"][0]
    return ctx, attn
